# revision 1
# baseline (speedup 1.0000x reference)
"""GraphWaveNet kernel for 8 Trainium2 NeuronCores.

Data-parallel over batch B=16 across 8 cores (2 batch elements per core);
adjacency and all weights are replicated, per the sharding hint. The full
forward pass is compiled per-core with XLA-Neuron and launched SPMD via
jax.pmap; outputs are gathered back to the full [B, N, O] shape.
"""

import numpy as np
import jax
import jax.numpy as jnp
from functools import partial

B, L, N, CIN = 16, 13, 1024, 2
NLAYERS = 8
BN_EPS = 1e-5
DILATIONS = (1, 2, 1, 2, 1, 2, 1, 2)
N_CORES = 8


def _conv1x1(w, b, h):
    # w: [Cout, Cin], h: [b, Cin, N, L]
    return jnp.einsum('oc,bcnl->bonl', w, h) + b[None, :, None, None]


def _forward(x, node_emb, start_w, start_b, tcn_w, tcn_b, skip_w, skip_b,
             gcn_w, gcn_b, bn_gamma, bn_beta, bn_mean, bn_var,
             end1_w, end1_b, end2_w, end2_b):
    # x: [b_shard, L, N, CIN]
    h = jnp.transpose(x, (0, 3, 2, 1))  # [b, CIN, N, L]
    adp = jax.nn.softmax(jax.nn.relu(node_emb @ node_emb.T), axis=1)  # [N, N]
    h = _conv1x1(start_w, start_b, h)  # [b, R, N, L]
    skip = None
    for i, d in enumerate(DILATIONS):
        residual = h
        Lc = h.shape[-1]
        conv = (jnp.einsum('oc,bcnl->bonl', tcn_w[i, :, :, 0], h[..., :Lc - d])
                + jnp.einsum('oc,bcnl->bonl', tcn_w[i, :, :, 1], h[..., d:])
                + tcn_b[i][None, :, None, None])
        g = jnp.tanh(conv) * jax.nn.sigmoid(conv)  # [b, D, N, Lc-d]
        # skip path only ever survives at the final time step, so slice first
        s_last = jnp.einsum('oc,bcn->bon', skip_w[i], g[..., -1]) + skip_b[i][None, :, None]
        skip = s_last if skip is None else s_last + skip
        if i == NLAYERS - 1:
            break  # last layer's diffusion output is dead code
        gd = jnp.einsum('bcvl,vw->bcwl', g, adp)
        gd = jax.nn.relu(_conv1x1(gcn_w[i], gcn_b[i], gd))  # [b, R, N, Lc-d]
        h = gd + residual[..., -gd.shape[-1]:]
        inv = bn_gamma[i] * jax.lax.rsqrt(bn_var[i] + BN_EPS)
        h = (h - bn_mean[i][None, :, None, None]) * inv[None, :, None, None] \
            + bn_beta[i][None, :, None, None]
    out = jax.nn.relu(skip)  # [b, S, N]
    out = jax.nn.relu(jnp.einsum('oc,bcn->bon', end1_w, out) + end1_b[None, :, None])
    out = jnp.einsum('oc,bcn->bon', end2_w, out) + end2_b[None, :, None]  # [b, O, N]
    return jnp.transpose(out, (0, 2, 1))  # [b, N, O]


_pmapped = None


def _get_pmapped():
    global _pmapped
    if _pmapped is None:
        devs = jax.devices()[:N_CORES]
        _pmapped = jax.pmap(
            _forward,
            in_axes=(0,) + (None,) * 17,
            devices=devs,
        )
    return _pmapped


def kernel(**inputs):
    x = np.asarray(inputs["x"], dtype=np.float32)
    xs = x.reshape(N_CORES, B // N_CORES, L, N, CIN)
    weight_names = [
        "node_emb", "start_w", "start_b", "tcn_w", "tcn_b", "skip_w", "skip_b",
        "gcn_w", "gcn_b", "bn_gamma", "bn_beta", "bn_mean", "bn_var",
        "end1_w", "end1_b", "end2_w", "end2_b",
    ]
    ws = [np.asarray(inputs[k], dtype=np.float32) for k in weight_names]
    fn = _get_pmapped()
    out = fn(xs, *ws)  # [8, 2, N, O]
    out = np.asarray(jax.device_get(out), dtype=np.float32)
    return out.reshape(B, N, out.shape[-1])


# revision 2
# speedup vs baseline: 1.7448x; 1.7448x over previous
"""GraphWaveNet kernel for 8 Trainium2 NeuronCores.

Data-parallel over batch B=16 across 8 cores (2 batch elements per core);
adjacency and all weights are replicated, per the sharding hint. The full
forward pass is compiled per-core with XLA-Neuron and launched SPMD via
jax.pmap; outputs are gathered back to the full [B, N, O] shape.
"""

import numpy as np
import jax
import jax.numpy as jnp
from functools import partial

B, L, N, CIN = 16, 13, 1024, 2
NLAYERS = 8
BN_EPS = 1e-5
DILATIONS = (1, 2, 1, 2, 1, 2, 1, 2)
N_CORES = 8


def _conv1x1(w, b, h):
    # w: [Cout, Cin], h: [b, Cin, N, L]
    return jnp.einsum('oc,bcnl->bonl', w, h) + b[None, :, None, None]


def _forward(x, node_emb, start_w, start_b, tcn_w, tcn_b, skip_w, skip_b,
             gcn_w, gcn_b, bn_gamma, bn_beta, bn_mean, bn_var,
             end1_w, end1_b, end2_w, end2_b):
    # x: [b_shard, L, N, CIN]
    h = jnp.transpose(x, (0, 3, 2, 1))  # [b, CIN, N, L]
    adp = jax.nn.softmax(jax.nn.relu(node_emb @ node_emb.T), axis=1)  # [N, N]
    h = _conv1x1(start_w, start_b, h)  # [b, R, N, L]
    skip = None
    for i, d in enumerate(DILATIONS):
        residual = h
        Lc = h.shape[-1]
        conv = (jnp.einsum('oc,bcnl->bonl', tcn_w[i, :, :, 0], h[..., :Lc - d])
                + jnp.einsum('oc,bcnl->bonl', tcn_w[i, :, :, 1], h[..., d:])
                + tcn_b[i][None, :, None, None])
        g = jnp.tanh(conv) * jax.nn.sigmoid(conv)  # [b, D, N, Lc-d]
        # skip path only ever survives at the final time step, so slice first
        s_last = jnp.einsum('oc,bcn->bon', skip_w[i], g[..., -1]) + skip_b[i][None, :, None]
        skip = s_last if skip is None else s_last + skip
        if i == NLAYERS - 1:
            break  # last layer's diffusion output is dead code
        gd = jnp.einsum('bcvl,vw->bcwl', g, adp)
        gd = jax.nn.relu(_conv1x1(gcn_w[i], gcn_b[i], gd))  # [b, R, N, Lc-d]
        h = gd + residual[..., -gd.shape[-1]:]
        inv = bn_gamma[i] * jax.lax.rsqrt(bn_var[i] + BN_EPS)
        h = (h - bn_mean[i][None, :, None, None]) * inv[None, :, None, None] \
            + bn_beta[i][None, :, None, None]
    out = jax.nn.relu(skip)  # [b, S, N]
    out = jax.nn.relu(jnp.einsum('oc,bcn->bon', end1_w, out) + end1_b[None, :, None])
    out = jnp.einsum('oc,bcn->bon', end2_w, out) + end2_b[None, :, None]  # [b, O, N]
    return jnp.transpose(out, (0, 2, 1))  # [b, N, O]


_pmapped = None
_weight_cache = {"key": None, "ws": None}

_WEIGHT_NAMES = [
    "node_emb", "start_w", "start_b", "tcn_w", "tcn_b", "skip_w", "skip_b",
    "gcn_w", "gcn_b", "bn_mean", "bn_var", "bn_gamma", "bn_beta",
    "end1_w", "end1_b", "end2_w", "end2_b",
]


def _get_pmapped():
    global _pmapped
    if _pmapped is None:
        devs = jax.devices()[:N_CORES]
        _pmapped = jax.pmap(
            lambda x, w: _forward(
                x, w["node_emb"], w["start_w"], w["start_b"], w["tcn_w"],
                w["tcn_b"], w["skip_w"], w["skip_b"], w["gcn_w"], w["gcn_b"],
                w["bn_gamma"], w["bn_beta"], w["bn_mean"], w["bn_var"],
                w["end1_w"], w["end1_b"], w["end2_w"], w["end2_b"]),
            in_axes=(0, 0),
            devices=devs,
        )
    return _pmapped


def _replicated_weights(inputs):
    """Place the replicated weight pytree on all cores once; reuse across calls."""
    ws = {k: np.ascontiguousarray(np.asarray(inputs[k], dtype=np.float32))
          for k in _WEIGHT_NAMES}
    key = tuple(hash(w.tobytes()) for w in ws.values())
    if _weight_cache["key"] != key:
        devs = jax.devices()[:N_CORES]
        _weight_cache["ws"] = jax.device_put_replicated(ws, devs)
        _weight_cache["key"] = key
    return _weight_cache["ws"]


def kernel(**inputs):
    x = np.asarray(inputs["x"], dtype=np.float32)
    xs = x.reshape(N_CORES, B // N_CORES, L, N, CIN)
    w_dev = _replicated_weights(inputs)
    fn = _get_pmapped()
    out = fn(xs, w_dev)  # [8, 2, N, O]
    out = np.asarray(jax.device_get(out), dtype=np.float32)
    return out.reshape(B, N, out.shape[-1])


# revision 3
# speedup vs baseline: 2.5062x; 1.4364x over previous
"""GraphWaveNet kernel for 8 Trainium2 NeuronCores.

Data-parallel over batch B=16 across 8 cores (2 batch elements per core);
adjacency and all weights are replicated, per the sharding hint. The full
forward pass is compiled per-core with XLA-Neuron and launched SPMD via
jax.pmap; outputs are gathered back to the full [B, N, O] shape.
"""

import numpy as np
import jax
import jax.numpy as jnp
from functools import partial

B, L, N, CIN = 16, 13, 1024, 2
NLAYERS = 8
BN_EPS = 1e-5
DILATIONS = (1, 2, 1, 2, 1, 2, 1, 2)
N_CORES = 8


def _conv1x1(w, b, h):
    # w: [Cout, Cin], h: [b, Cin, N, L]
    return jnp.einsum('oc,bcnl->bonl', w, h) + b[None, :, None, None]


def _forward(x, node_emb, start_w, start_b, tcn_w, tcn_b, skip_w, skip_b,
             gcn_w, gcn_b, bn_gamma, bn_beta, bn_mean, bn_var,
             end1_w, end1_b, end2_w, end2_b):
    # x: [b_shard, L, N, CIN]
    h = jnp.transpose(x, (0, 3, 2, 1))  # [b, CIN, N, L]
    adp = jax.nn.softmax(jax.nn.relu(node_emb @ node_emb.T), axis=1)  # [N, N]
    h = _conv1x1(start_w, start_b, h)  # [b, R, N, L]
    skip = None
    for i, d in enumerate(DILATIONS):
        residual = h
        Lc = h.shape[-1]
        conv = (jnp.einsum('oc,bcnl->bonl', tcn_w[i, :, :, 0], h[..., :Lc - d])
                + jnp.einsum('oc,bcnl->bonl', tcn_w[i, :, :, 1], h[..., d:])
                + tcn_b[i][None, :, None, None])
        g = jnp.tanh(conv) * jax.nn.sigmoid(conv)  # [b, D, N, Lc-d]
        # skip path only ever survives at the final time step, so slice first
        s_last = jnp.einsum('oc,bcn->bon', skip_w[i], g[..., -1]) + skip_b[i][None, :, None]
        skip = s_last if skip is None else s_last + skip
        if i == NLAYERS - 1:
            break  # last layer's diffusion output is dead code
        # bf16 operands with fp32 accumulation: 4x TensorE rate vs fp32
        gd = jnp.einsum('bcvl,vw->bcwl',
                        g.astype(jnp.bfloat16), adp.astype(jnp.bfloat16),
                        preferred_element_type=jnp.float32)
        gd = jax.nn.relu(_conv1x1(gcn_w[i], gcn_b[i], gd))  # [b, R, N, Lc-d]
        h = gd + residual[..., -gd.shape[-1]:]
        inv = bn_gamma[i] * jax.lax.rsqrt(bn_var[i] + BN_EPS)
        h = (h - bn_mean[i][None, :, None, None]) * inv[None, :, None, None] \
            + bn_beta[i][None, :, None, None]
    out = jax.nn.relu(skip)  # [b, S, N]
    out = jax.nn.relu(jnp.einsum('oc,bcn->bon', end1_w, out) + end1_b[None, :, None])
    out = jnp.einsum('oc,bcn->bon', end2_w, out) + end2_b[None, :, None]  # [b, O, N]
    return jnp.transpose(out, (0, 2, 1))  # [b, N, O]


_pmapped = None
_weight_cache = {"key": None, "ws": None}

_WEIGHT_NAMES = [
    "node_emb", "start_w", "start_b", "tcn_w", "tcn_b", "skip_w", "skip_b",
    "gcn_w", "gcn_b", "bn_mean", "bn_var", "bn_gamma", "bn_beta",
    "end1_w", "end1_b", "end2_w", "end2_b",
]


def _get_pmapped():
    global _pmapped
    if _pmapped is None:
        devs = jax.devices()[:N_CORES]
        _pmapped = jax.pmap(
            lambda x, w: _forward(
                x, w["node_emb"], w["start_w"], w["start_b"], w["tcn_w"],
                w["tcn_b"], w["skip_w"], w["skip_b"], w["gcn_w"], w["gcn_b"],
                w["bn_gamma"], w["bn_beta"], w["bn_mean"], w["bn_var"],
                w["end1_w"], w["end1_b"], w["end2_w"], w["end2_b"]),
            in_axes=(0, 0),
            devices=devs,
        )
    return _pmapped


def _replicated_weights(inputs):
    """Place the replicated weight pytree on all cores once; reuse across calls."""
    ws = {k: np.ascontiguousarray(np.asarray(inputs[k], dtype=np.float32))
          for k in _WEIGHT_NAMES}
    key = tuple(hash(w.tobytes()) for w in ws.values())
    if _weight_cache["key"] != key:
        devs = jax.devices()[:N_CORES]
        _weight_cache["ws"] = jax.device_put_replicated(ws, devs)
        _weight_cache["key"] = key
    return _weight_cache["ws"]


def kernel(**inputs):
    x = np.asarray(inputs["x"], dtype=np.float32)
    xs = x.reshape(N_CORES, B // N_CORES, L, N, CIN)
    w_dev = _replicated_weights(inputs)
    fn = _get_pmapped()
    out = fn(xs, w_dev)  # [8, 2, N, O]
    out = np.asarray(jax.device_get(out), dtype=np.float32)
    return out.reshape(B, N, out.shape[-1])
